# revision 43
# baseline (speedup 1.0000x reference)
"""Trainium2 Bass kernel for the BinaryLayer problem.

Math: out[b,o] = OR_r ( mask[o,r] AND AND_t x_in[b, w[o,r,t]] ) with
x_in = [1 | x | 1-x].  AND over 16 literals == (sum of literal values == 16).
sum_t lit = base[j] + sum_f C[f,j]*x[b,f]  where for j=(o,r):
  C[f,j]  = (#slots with w==f+1) - (#slots with w==f+1+F)
  base[j] = (#slots with w==0) + (#slots with w>F)
Fold threshold+mask into a constant row: c1[j] = base[j]-16 (valid term)
or base[j]-64 (padded term, all w==0).  Then with S[b,j] = x_aug[b,:]@A[:,j]
(A = [C; c1], x_aug = [x, 1]):  AND true <=> S==0, and since S<=0 always,
out[b,o] = (max_r S[b,o*32+r] == 0).  All arithmetic is exact small-int in
fp8e4m3 inputs / f32 PSUM accumulation; S in [-64, 0] so the max is emitted
as int8 and the ==0 test runs on the host during unsharding.

Sharding: 2D - 4 batch groups x 2 column halves across 8 cores.  Per core
the host packs one fp8 tensor D [785, 3072] = [x^T half0 | A block0..3 |
x^T half1] - column order == consumption order - so the whole input ships
in six >=1KB-element DMAs (every DMA costs ~625ns of serialized HWDGE
issue, so transfer count matters as much as bytes).  k = s*128 + p over 8
subtiles; the 17-row k-tail lands via one tiny DMA, the rest of subtiles
6/7 zeroed by early memsets on DVE/Pool (regions disjoint from the tail
DMA so no ordering).

Schedule: all matmuls fp8 DoubleRow (4 passes of 2 subtiles, 107ns per
512-col matmul warm); dummy matmuls on zeroed scratch bridge the ~3us PE
pstate ramp under the first loads.  Rounds (column block x batch tiles)
run the low batch half first across all column blocks so x's second half
is the last DMA; jb2/jb3 high halves are single-bank rounds so the final
reduces are short.  Each round max-reduces its PSUM banks over the 32
or-terms via mixed two-engine paths (per 2-bank round: 'dve' DVE
tensor_reduce 1192ns; 'act' ACT copy->bf16 997 + DVE 2x bf16 tree 816;
'pooldve' gpsimd first tree level 806 + DVE reduce 593; 'poolfull' gpsimd
5-level tree 1854), with the PSUM-freeing stage emitted in round order
and tails one round later so bank recycling never waits a full tree.
Outputs leave as two int8 DMAs.
"""

import os

os.environ.setdefault("MYCRO_LOCAL_CACHE", "1")

import numpy as np

import concourse.bass as bass
import concourse.bacc as bacc
import concourse.mybir as mybir
from concourse.tile import TileContext
from concourse.bass_utils import run_bass_kernel_spmd

B, F = 4096, 784
OUT, OR_T, AND_T = 128, 32, 16
N_CORES = 8
BG, JG = 4, 2                # batch groups x column halves
BS = B // BG                 # 1024 batch rows per core
J = OUT * OR_T // JG         # 2048 (o,r) columns per core, j = o*32 + r
OJ = OUT // JG               # 64 output features per core
K = F + 1                    # 785 = 784 features + constant row
KMAIN = 768                  # rows in the six full 128-row k-subtiles
KTAIL = K - KMAIN            # 17-row tail in subtile 6
NSUB = 8                     # 6 full + tail + zero pad (DoubleRow pairing)
NBT = BS // 128              # 8 batch tiles per core
NJB = J // 512               # 4 column blocks
W = 512 * (NJB + 2)          # 3072 packed columns: x half0 | A jb0..3 | x half1
FP8 = mybir.dt.float8e4
FP8_NP = mybir.dt.np(FP8)
DR = mybir.MatmulPerfMode.DoubleRow
MAX = mybir.AluOpType.max

# Packed-column offsets inside D / the SBUF tile.
XL = slice(0, 512)            # x columns, batch tiles 0-3
XH = slice(2560, 3072)        # x columns, batch tiles 4-7
AJ = [slice(512 * (jb + 1), 512 * (jb + 2)) for jb in range(NJB)]

# Rounds: (column block, batch tiles, reduce mode).
# Hardware constraints: GPSIMD/Pool cannot touch PSUM and supports no
# max op at all (only add), and no instruction may read two PSUM
# operands.  So each PSUM bank is drained either by a full DVE
# tensor_reduce max ('dve', 1192ns/2banks, y = max_r S, true <=> 0) or
# by ACT computing relu(S+1) indicators into bf16 ('act'/'actpool',
# 1038) whose or-group COUNT is then summed by a DVE 2x bf16 first
# tree level (327) plus an add-tail on DVE ('act', 489) or Pool
# ('actpool', 1047); for those units y = count, true <=> y != 0.  The
# host applies the per-unit rule while unsharding.  Assignment keeps
# ACT and DVE PSUM-draining balanced and Pool absorbing tails.
ROUNDS = [
    (0, [0, 1], "actpool"), (0, [2, 3], "dve"),
    (1, [0, 1], "actpool"), (1, [2, 3], "dve"),
    (2, [0, 1], "actpool"), (2, [2, 3], "dve"),
    (3, [0, 1], "actpool"), (3, [2, 3], "dve"),
    (0, [4, 5], "actpool"), (0, [6, 7], "dve"),
    (1, [4, 5], "actpool"), (1, [6, 7], "dve"),
    (2, [4], "actpool"), (2, [5], "dve"), (2, [6], "actpool"), (2, [7], "dve"),
    (3, [4], "actpool"), (3, [5], "dve"), (3, [6], "actpool"), (3, [7], "dve"),
]

_CACHE: dict = {}


def _build_nc(use_double_row: bool = True) -> bass.Bass:
    nc = bacc.Bacc("TRN2")
    D_d = nc.declare_dram_parameter("D", [K, W], FP8, isOutput=False)
    y_d = nc.declare_dram_parameter("y", [BS, OJ], mybir.dt.bfloat16, isOutput=True)
    f32 = mybir.dt.float32
    bf16 = mybir.dt.bfloat16

    with TileContext(nc) as tc:
        with (
            tc.tile_pool(name="const", bufs=1) as cpool,
            tc.tile_pool(name="psum", bufs=4, space="PSUM") as ppool,
            tc.tile_pool(name="ind", bufs=3) as ipool,
            tc.tile_pool(name="scf", bufs=3) as fpool,
            tc.tile_pool(name="scb", bufs=3) as bpool,
        ):
            # One packed SBUF tile holds x and A in DRAM column order.
            d_sb = cpool.tile([128, NSUB, W], FP8)
            y_i = cpool.tile([128, NBT, OJ], mybir.dt.bfloat16)
            wu_sb = cpool.tile([128, 2, 704], FP8)
            ps_w = ppool.tile([128, 2, 16, 32], f32, name="ps", tag="ps")
            nc.gpsimd.memset(wu_sb[:, :, 0:192], 0.0)
            nc.gpsimd.memset(wu_sb[:, :, 192:704], 0.0)

            # Zero the k-tail pair (subtiles 6 and 7); the 17-row tail DMA
            # then overwrites the real rows of subtile 6 (Tile orders it
            # after these).  Early columns (needed by round 0) on DVE, the
            # late half on Pool - both engines are idle this early.
            nc.vector.memset(d_sb[:, 6, 0:1536], 0.0)
            nc.vector.memset(d_sb[:, 7, 0:1536], 0.0)
            nc.gpsimd.memset(d_sb[:, 6, 1536:3072], 0.0)
            nc.gpsimd.memset(d_sb[:, 7, 1536:3072], 0.0)

            # Six input DMAs in consumption order.
            def load(csl):
                nc.sync.dma_start(
                    out=d_sb[:, 0:6, csl],
                    in_=D_d[0:KMAIN, csl].rearrange("(s p) j -> p s j", p=128),
                )

            nc.sync.dma_start(
                out=d_sb[:, 0:2, 0:1024],
                in_=D_d[0:256, 0:1024].rearrange("(s p) j -> p s j", p=128),
            )
            nc.sync.dma_start(out=d_sb[0:KTAIL, 6, :], in_=D_d[KMAIN:K, :])
            nc.sync.dma_start(
                out=d_sb[:, 2:6, 0:1024],
                in_=D_d[256:KMAIN, 0:1024].rearrange("(s p) j -> p s j", p=128),
            )
            for jb in range(1, NJB):
                load(AJ[jb])
            load(XH)

            def warm(n_small, n_big):
                for _ in range(n_small):
                    nc.tensor.matmul(
                        ps_w[:, 0, 0:2, :], wu_sb[:, :, 0:128],
                        wu_sb[:, :, 128:192],
                        start=True, stop=True, perf_mode=DR,
                    )
                for _ in range(n_big):
                    nc.tensor.matmul(
                        ps_w[:, 0], wu_sb[:, :, 0:128], wu_sb[:, :, 192:704],
                        start=True, stop=True, perf_mode=DR,
                    )

            warm(30, 6)

            n_sp = 4 if use_double_row else NSUB

            def emit_matmuls(ps, bts, jsl, sp_order=None, pads=None):
                sps = list(sp_order or range(n_sp))
                last_sp = sps[-1]
                for k, sp in enumerate(sps):
                    if pads and pads[k]:
                        warm(0, pads[k])
                    for i, bt in enumerate(bts):
                        xs = XL if bt < 4 else XH
                        bsl = slice(xs.start + (bt % 4) * 128,
                                    xs.start + (bt % 4 + 1) * 128)
                        if use_double_row:
                            ssl = slice(2 * sp, 2 * sp + 2)
                            nc.tensor.matmul(
                                ps[:, i], d_sb[:, ssl, bsl], d_sb[:, ssl, jsl],
                                start=(sp == 0), stop=(sp == last_sp),
                                perf_mode=DR,
                            )
                        else:
                            nc.tensor.matmul(
                                ps[:, i], d_sb[:, sp, bsl], d_sb[:, sp, jsl],
                                start=(sp == 0), stop=(sp == last_sp),
                            )

            ADD = mybir.AluOpType.add

            def tree_tail(eng, dst, sc, nb):
                # pairwise-add tail over indicator counts (Pool has no max)
                s = sc[:, 0:nb]
                for h in (8, 4, 2):
                    eng.tensor_tensor(s[:, :, :, 0:h], s[:, :, :, 0:h],
                                      s[:, :, :, h : 2 * h], ADD)
                eng.tensor_tensor(dst, s[:, :, :, 0], s[:, :, :, 1], ADD)

            deferred = []
            for r, (jb, bts, mode) in enumerate(ROUNDS):
                jsl = AJ[jb]
                nb = len(bts)
                ps = ppool.tile([128, 2, 16, 32], f32, name="ps", tag="ps")
                psn = ps[:, 0:nb]
                # Round 0 runs its k-tail pass second (the tail DMA is tiny
                # and lands before the bulk of the first block), with warm
                # matmuls bridging the wait for that bulk.
                if r == 0 and n_sp == 4:
                    emit_matmuls(ps, bts, jsl, sp_order=[0, 3, 1, 2],
                                 pads=[0, 0, 5, 0])
                else:
                    emit_matmuls(ps, bts, jsl)
                out_ap = y_i[:, bts[0] : bts[0] + nb, jb * 16 : (jb + 1) * 16]
                # PSUM-freeing stage now; tail deferred one round.
                if mode == "dve":
                    nc.vector.tensor_reduce(
                        out=out_ap, in_=psn,
                        axis=mybir.AxisListType.X, op=MAX,
                    )
                else:  # 'act' / 'actpool': relu(S+1) indicators, then count
                    ind = ipool.tile([128, 2, 16, 32], bf16, name="ind", tag="ind")
                    nc.scalar.activation(
                        out=ind[:, 0:nb], in_=psn,
                        func=mybir.ActivationFunctionType.Relu,
                        bias=1.0,
                    )
                    tail_eng = nc.vector if mode == "act" else nc.gpsimd

                    def t_ind(o=out_ap, i=ind, n=nb, eng=tail_eng):
                        sc = bpool.tile([128, 2, 16, 16], bf16,
                                        name="sb", tag="sb")
                        with nc.allow_low_precision(reason="counts<=32 exact in bf16"):
                            nc.vector.tensor_tensor(
                                sc[:, 0:n], i[:, 0:n, :, 0:16],
                                i[:, 0:n, :, 16:32], ADD)
                            tree_tail(eng, o, sc, n)

                    deferred.append(t_ind)
                while len(deferred) > 1:
                    deferred.pop(0)()
            for fn_ in deferred:
                fn_()
            for g in range(2):
                nc.sync.dma_start(
                    out=y_d[g * 512 : (g + 1) * 512, :].rearrange(
                        "(t p) o -> p t o", p=128
                    ),
                    in_=y_i[:, g * 4 : (g + 1) * 4, :],
                )
    return nc


def _get_nc() -> bass.Bass:
    if "nc" not in _CACHE:
        nc = _build_nc(use_double_row=_CACHE.get("dr", True))
        nc.finalize()
        _CACHE["nc"] = nc
    return _CACHE["nc"]


def _build_A(weights: np.ndarray) -> np.ndarray:
    JF = OUT * OR_T
    w = weights.reshape(JF, AND_T).astype(np.int64)
    v = w.reshape(-1)
    j_idx = np.repeat(np.arange(JF), AND_T)
    C = np.zeros((K, JF), np.float32)
    pos = (v >= 1) & (v <= F)
    neg = v > F
    np.add.at(C, (v[pos] - 1, j_idx[pos]), 1.0)
    np.add.at(C, (v[neg] - 1 - F, j_idx[neg]), -1.0)
    base = (w == 0).sum(1) + neg.reshape(JF, AND_T).sum(1)
    padded = (w == 0).all(1)
    C[F, :] = np.where(padded, base - 64.0, base - 16.0).astype(np.float32)
    A8 = C.astype(FP8_NP)
    assert np.array_equal(A8.astype(np.float32), C), "fp8 must be exact"
    return A8


def _make_in_maps(x: np.ndarray, weights: np.ndarray) -> list[dict]:
    A_full = _build_A(weights)
    xT = np.zeros((K, B), FP8_NP)
    xT[:F] = (x.T != 0).astype(FP8_NP)
    xT[F] = 1.0
    in_maps = []
    for c in range(N_CORES):
        bg, jg = divmod(c, JG)
        xc = xT[:, bg * BS : (bg + 1) * BS]
        Ac = A_full[:, jg * J : (jg + 1) * J]
        D = np.concatenate([xc[:, 0:512], Ac, xc[:, 512:1024]], axis=1)
        in_maps.append({"D": np.ascontiguousarray(D)})
    return in_maps


def _count_mask() -> np.ndarray:
    # Per-core [BS, OJ] mask: True where the unit emitted an or-term COUNT
    # (true <=> y != 0), False where it emitted max_r S (true <=> y == 0).
    m = np.zeros((NBT, NJB), dtype=bool)
    for jb, bts, mode in ROUNDS:
        for bt in bts:
            m[bt, jb] = mode != "dve"
    return np.repeat(np.repeat(m, 128, axis=0), 16, axis=1)


def kernel(x: np.ndarray, weights: np.ndarray) -> np.ndarray:
    x = np.asarray(x)
    weights = np.asarray(weights)
    in_maps = _make_in_maps(x, weights)
    nc = _get_nc()
    res = run_bass_kernel_spmd(nc, in_maps, list(range(N_CORES)))
    cm = _count_mask()
    y = np.empty((B, OUT), dtype=bool)
    for c in range(N_CORES):
        bg, jg = divmod(c, JG)
        raw = np.asarray(res.results[c]["y"]).astype(np.float32)
        y[bg * BS : (bg + 1) * BS, jg * OJ : (jg + 1) * OJ] = np.where(
            cm, raw != 0, raw == 0
        )
    return y
